# revision 31
# baseline (speedup 1.0000x reference)
"""BERT-base (12-layer, B=8, T=512, D=768) forward + tied-embedding LM head
on 8 Trainium2 NeuronCores.

Sharding: data-parallel over the batch dimension -- core b computes batch
element b end-to-end (no collectives).

v2: fp8e4 DoubleRow matmuls for the five big GEMMs (qk, v, proj, fc1, fc2)
with power-of-two scales folded into evacuation ops; attention uses a
transposed AV formulation (out = p.T @ [ones|v], full-128 contraction) so
softmax normalization is a per-partition DVE reciprocal + scalar multiply --
no Ln/Exp act-table thrash, no DRAM broadcast round-trips. Residual stream
kept in bf16 so LN stat matmuls run at 1 cycle/row. Logits emitted bf16.
"""

from contextlib import ExitStack

import numpy as np
import ml_dtypes

import concourse.bass as bass
import concourse.bacc as bacc
import concourse.mybir as mybir
import concourse.tile as tile
from concourse import bass_utils
from concourse._compat import get_trn_type

NP_BF16 = ml_dtypes.bfloat16
NP_FP8 = ml_dtypes.float8_e4m3

FP32 = mybir.dt.float32
BF16 = mybir.dt.bfloat16
FP8 = mybir.dt.float8e4
AF = mybir.ActivationFunctionType
OP = mybir.AluOpType
DR = mybir.MatmulPerfMode.DoubleRow

P = 128
T = 512
D = 768
H = 12
HD = 64
DF = 3072
V = 30522
DK = D // P       # 6 feature chunks
KP = DK // 2      # 3 DoubleRow k-pairs over D
FKP = DF // 256   # 12 DoubleRow k-pairs over DF
TCH = T // P      # 4 token chunks
EPS = 1e-5

# power-of-two quantization scales
SA = 16.0         # LN output (h) fp8 scale
SW_QKV = 64.0     # wqk / wv / wfc weight scale
SW_PR = 256.0     # wproj / wfc2 weight scale
SV = 32.0         # v fp8 scale (folded: v8 = psum * SV/(SA*SW_QKV))
SP = 16.0         # softmax exp fp8 scale (folded into exp bias as ln SP)
EXP_SCALE = 0.125 / (SA * SW_QKV) ** 2       # 2^-23
V_EVAC = SV / (SA * SW_QKV)                  # 2^-5
PR_EVAC = 1.0 / (SV * SW_PR)                 # 2^-13
FC1_EVAC = 1.0 / (SA * SW_QKV)               # 2^-10
FC2_EVAC = 1.0 / SW_PR                       # 2^-8
LN_SA_BIAS = float(np.log(SA))


def build(n_layers=12, with_head=True, debug_taps=()):
    nc = bacc.Bacc(get_trn_type() or "TRN2", target_bir_lowering=False, debug=False)

    x0T = nc.dram_tensor("x0T", [D, T], BF16, kind="ExternalInput")
    mb = nc.dram_tensor("mb", [P, TCH], FP32, kind="ExternalInput")
    ident_d = nc.dram_tensor("ident", [P, P], BF16, kind="ExternalInput")
    L = max(n_layers, 1)
    wqk = nc.dram_tensor("wqk", [L, D, 2 * D], FP8, kind="ExternalInput")
    wv = nc.dram_tensor("wv", [L, D, D], FP8, kind="ExternalInput")
    wpr = nc.dram_tensor("wpr", [L, D, D], FP8, kind="ExternalInput")
    wfc = nc.dram_tensor("wfc", [L, D, DF], FP8, kind="ExternalInput")
    wf2 = nc.dram_tensor("wf2", [L, DF, D], FP8, kind="ExternalInput")
    if with_head:
        NVCH = (V + 511) // 512
        wembP = nc.dram_tensor("wembP", [NVCH, P, DK * 512], BF16,
                               kind="ExternalInput")
        out = nc.dram_tensor("out", [T, V], BF16, kind="ExternalOutput")
    else:
        out = nc.dram_tensor("out", [D, T], FP32, kind="ExternalOutput")

    tap_specs = {
        "h1": ([KP * P, 2 * T], FP8), "qkT": ([2 * D, T], BF16),
        "v66": ([2 * P, 2 * H * 66], FP8), "p0": ([P, 4 * T], FP8),
        "yT": ([KP * P, 2 * T], FP8), "xattn": ([D, T], BF16),
        "h2": ([KP * P, 2 * T], FP8), "gT": ([FKP * P, 2 * T], FP8),
        "xfinal": ([D, T], BF16),
    }
    taps = {}
    for name in debug_taps:
        shape, dt = tap_specs[name]
        taps[name] = nc.dram_tensor(f"tap_{name}", shape, dt, kind="ExternalOutput")

    with tile.TileContext(nc) as tc, ExitStack() as ctx:
        consts = ctx.enter_context(tc.tile_pool(name="consts", bufs=1))
        resid = ctx.enter_context(tc.tile_pool(name="resid", bufs=1))
        acts = ctx.enter_context(tc.tile_pool(name="acts", bufs=2))
        wpool = ctx.enter_context(tc.tile_pool(name="wpool", bufs=1))
        small = ctx.enter_context(tc.tile_pool(name="small", bufs=2))
        ps_ln = ctx.enter_context(tc.tile_pool(name="ps_ln", bufs=1, space="PSUM"))
        ps_gemm = ctx.enter_context(tc.tile_pool(name="ps_gemm", bufs=1, space="PSUM"))
        ps_attn = ctx.enter_context(tc.tile_pool(name="ps_attn", bufs=1, space="PSUM"))

        czero = consts.tile([P, 1], FP32, tag="czero")
        nc.vector.memset(czero[:], 0.0)
        ceps = consts.tile([P, 1], FP32, tag="ceps")
        nc.vector.memset(ceps[:], EPS)
        nc.const_aps.aps[(FP32, 0.0)] = czero[:]
        nc.const_aps.aps[(FP32, EPS)] = ceps[:]
        csa = consts.tile([P, 1], FP32, tag="csa")
        nc.vector.memset(csa[:], LN_SA_BIAS)
        nc.const_aps.aps[(FP32, LN_SA_BIAS)] = csa[:]

        ones_bf = consts.tile([P, 1], BF16, tag="ones_bf")
        nc.vector.memset(ones_bf[:], 1.0)
        ones_row = consts.tile([1, P], BF16, tag="ones_row")
        nc.vector.memset(ones_row[:], 1.0)
        mb_sb = consts.tile([P, TCH], FP32, tag="mb_sb")
        nc.sync.dma_start(mb_sb[:], mb[:])
        ident = consts.tile([P, P], BF16, tag="ident")
        nc.sync.dma_start(ident[:], ident_d[:])

        ones_f32 = consts.tile([P, 1], FP32, tag="ones_f32")
        nc.vector.memset(ones_f32[:], 1.0)
        F32R = mybir.dt.float32r

        # residual stream, bf16 feature-major [D, T] in 6 chunks
        xT = []
        for j in range(DK):
            t = resid.tile([P, T], BF16, tag=f"x{j}")
            nc.sync.dma_start(t[:], x0T[j * P:(j + 1) * P, :])
            xT.append(t)

        def layer_norm(tag):
            """LN over xT -> 3 fp8 pair-tiles [P, 2, T], values scaled by SA"""
            sum_ps = ps_ln.tile([1, T], FP32, tag="stat", bufs=2, name="sum_ps")
            ssq_ps = ps_ln.tile([1, T], FP32, tag="stat", bufs=2, name="ssq_ps")
            sq_tiles = []
            for j in range(DK):
                sq = acts.tile([P, T], BF16, tag="sq", bufs=6)
                nc.gpsimd.tensor_mul(sq[:], xT[j][:], xT[j][:])
                sq_tiles.append(sq)
                nc.tensor.matmul(sum_ps[:], ones_bf[:], xT[j][:],
                                 start=(j == 0), stop=(j == DK - 1))
            for j in range(DK):
                nc.tensor.matmul(ssq_ps[:], ones_bf[:], sq_tiles[j][:],
                                 start=(j == 0), stop=(j == DK - 1))
            nm = small.tile([1, T], FP32, tag="srow", bufs=4, name="nm")
            nc.vector.tensor_scalar_mul(nm[:], sum_ps[:], -1.0 / D)
            msq = small.tile([1, T], FP32, tag="srow", bufs=4, name="msq")
            nc.vector.tensor_mul(msq[:], nm[:], nm[:])
            var = small.tile([1, T], FP32, tag="srow", bufs=4, name="var")
            nc.vector.scalar_tensor_tensor(
                out=var[:], in0=ssq_ps[:], scalar=1.0 / D, in1=msq[:],
                op0=OP.mult, op1=OP.subtract)
            lnv = small.tile([1, T], FP32, tag="srow", bufs=4, name="lnv")
            nc.scalar.activation(lnv[:], var[:], AF.Ln, bias=EPS)
            # SA * rsqrt(var): exp(-0.5*ln(v) + ln SA), bf16 out
            rstd = small.tile([1, T], BF16, tag="brow", bufs=4, name="rstd")
            nc.scalar.activation(rstd[:], lnv[:], AF.Exp, scale=-0.5,
                                 bias=LN_SA_BIAS)
            nmrs = small.tile([1, T], BF16, tag="brow", bufs=4, name="nmrs")
            nc.vector.tensor_mul(nmrs[:], nm[:], rstd[:])
            rb = ps_ln.tile([P, T], FP32, tag="stat", bufs=2, name="rb")
            nc.tensor.matmul(rb[:], ones_row[:], rstd[:], start=True, stop=True)
            nb = ps_ln.tile([P, T], FP32, tag="stat", bufs=2, name="nb")
            nc.tensor.matmul(nb[:], ones_row[:], nmrs[:], start=True, stop=True)
            h_pairs = []
            for jp in range(KP):
                hp = acts.tile([P, 2, T], FP8, tag=f"h_{tag}", bufs=4)
                for j2 in range(2):
                    j = 2 * jp + j2
                    t1 = acts.tile([P, T], BF16, tag="lnt", bufs=3)
                    nc.vector.tensor_mul(t1[:], xT[j][:], rb[:])
                    nc.vector.tensor_add(hp[:, j2, :], t1[:], nb[:])
                h_pairs.append(hp)
            return h_pairs

        def gemm_fm(w3, l, M, rhs_pairs, tag, CG, evac, nkp=KP, evac_rot=1,
                    dma_q=None):
            """feature-major DoubleRow GEMM: psum[128,T] = sum_kp
            w3[l, kp-pair, m-chunk].T @ rhs_pairs[kp]"""
            ei = 0
            for cg0 in range(0, M, CG):
                cgn = min(CG, M - cg0)
                slabs = []
                for kp in range(nkp):
                    s = wpool.tile([P, 2, CG], FP8, tag=f"{tag}_{kp}", bufs=2)
                    eng = dma_q(kp) if dma_q else nc.sync
                    eng.dma_start(
                        s[:, :, :cgn],
                        w3[l, kp * 256:(kp + 1) * 256, cg0:cg0 + cgn]
                        .rearrange("(j p) c -> p j c", p=P))
                    slabs.append(s)
                for mi in range(cgn // P):
                    m = (cg0 // P) + mi
                    ps = ps_gemm.tile([P, T], FP32, tag="g", bufs=2)
                    for kp in range(nkp):
                        nc.tensor.matmul(ps[:], slabs[kp][:, :, mi * P:(mi + 1) * P],
                                         rhs_pairs[kp][:],
                                         start=(kp == 0), stop=(kp == nkp - 1),
                                         perf_mode=DR)
                    evac(m, ps, ei % evac_rot)
                    ei += 1

        def dump_pairs(name, tiles):
            if name in taps:
                for j, t in enumerate(tiles):
                    nc.sync.dma_start(
                        taps[name][j * P:(j + 1) * P, :],
                        t[:].rearrange("p j t -> p (j t)"))

        def layer(l):
            h1 = layer_norm("ln1")
            dump_pairs("h1", h1)

            # ---- QK gemm: qkT[c, t] raw (scaled by SA*SW_QKV), bf16 ----
            qkT = [None] * (2 * D // P)

            def qk_evac(m, ps, r):
                qt = acts.tile([P, T], BF16, tag="qkT", bufs=12)
                if r == 0:
                    nc.scalar.copy(qt[:], ps[:])
                else:
                    nc.vector.tensor_copy(qt[:], ps[:])
                qkT[m] = qt
            gemm_fm(wqk, l, 2 * D, h1, "wqk", 2 * D, qk_evac, evac_rot=2,
                    dma_q=lambda kp: nc.sync)
            if "qkT" in taps:
                for j, t in enumerate(qkT):
                    nc.sync.dma_start(taps["qkT"][j * P:(j + 1) * P, :], t[:])

            # ---- V gemm (token-major): v66[kcp][p, j, h, 0]=1, [.,1:65]=v*SV
            v_slabs = []
            for kp in range(KP):
                s = wpool.tile([P, 2, D], FP8, tag=f"wv_{kp}", bufs=2)
                nc.sync.dma_start(
                    s[:], wv[l, kp * 256:(kp + 1) * 256, :]
                    .rearrange("(j p) c -> p j c", p=P))
                v_slabs.append(s)
            v66 = []
            for kcp in range(2):
                vt = acts.tile([P, 2, H, 66], FP8, tag="v66", bufs=4)
                nc.vector.memset(vt[:, :, :, 0:1], 1.0)
                v66.append(vt)

            def v_chunk(tch):
                for n0 in range(0, D, T):  # {0:512, 512:256}
                    nn = min(T, D - n0)
                    ps = ps_gemm.tile([P, T], FP32, tag="g", bufs=2)
                    for kp in range(KP):
                        nc.tensor.matmul(
                            ps[:, :nn],
                            h1[kp][:, :, tch * P:(tch + 1) * P],
                            v_slabs[kp][:, :, n0:n0 + nn],
                            start=(kp == 0), stop=(kp == KP - 1), perf_mode=DR)
                    dst = v66[tch // 2][:, tch % 2, n0 // HD:(n0 + nn) // HD, 1:65]
                    src = ps[:, :nn].rearrange("p (h d) -> p h d", d=HD)
                    if n0 == 0:
                        nc.scalar.activation(dst, src, AF.Copy, scale=V_EVAC)
                    else:
                        nc.vector.tensor_scalar_mul(dst, src, V_EVAC)

            # ---- attention: p = exp(k.T q * s + b) in fp8; y via p.T@[1|v]
            yT = [acts.tile([P, 2, T], FP8, tag="yT", bufs=4, name=f"yT{i}")
                  for i in range(KP)]
            p_tiles = {}

            def scores(g):
                for hh in range(4):
                    h = 4 * g + hh
                    ht, r = h // 2, h % 2
                    rows = slice(r * HD, (r + 1) * HD)
                    qt = qkT[ht]
                    kt = qkT[DK + ht]
                    for kcp in range(2):
                        pt = acts.tile([P, 2, T], FP8, tag="p", bufs=12)
                        for j2 in range(2):
                            kc = 2 * kcp + j2
                            s_ps = ps_attn.tile([P, T], FP32, tag="sc", bufs=2)
                            nc.tensor.matmul(s_ps[:], kt[rows, kc * P:(kc + 1) * P],
                                             qt[rows, :], start=True, stop=True)
                            nc.scalar.activation(pt[:, j2, :], s_ps[:], AF.Exp,
                                                 bias=mb_sb[:, kc:kc + 1],
                                                 scale=EXP_SCALE)
                        p_tiles[(h, kcp)] = pt
                if g == 0 and "p0" in taps:
                    for kcp in range(2):
                        nc.sync.dma_start(
                            taps["p0"][:, kcp * 2 * T:(kcp + 1) * 2 * T],
                            p_tiles[(0, kcp)][:].rearrange("p j t -> p (j t)"))

            def av_norm(g):
                for qc in range(TCH):
                    av = ps_attn.tile([P, 4, 66], FP32, tag="av", bufs=2,
                                      name="av")
                    for hh in range(4):
                        h = 4 * g + hh
                        for kcp in range(2):
                            nc.tensor.matmul(
                                av[:, hh, :],
                                p_tiles[(h, kcp)][:, :, qc * P:(qc + 1) * P],
                                v66[kcp][:, :, h, :],
                                start=(kcp == 0), stop=(kcp == 1), perf_mode=DR)
                    rec = small.tile([P, 4, 1], FP32, tag="rec", bufs=4)
                    nc.vector.reciprocal_approx_fast(rec[:], av[:, :, 0:1])
                    # one batched multiply: yt_all[p,hh,d] = av[p,hh,1+d]*rec[p,hh]
                    yt_all = acts.tile([P, 4, HD], BF16, tag="yt", bufs=4)
                    rec_b = bass.AP(tensor=rec.tensor, offset=rec.offset,
                                    ap=[rec[:].ap[0], [1, 4], [0, HD]])
                    nc.vector.tensor_mul(yt_all[:], av[:, :, 1:65], rec_b)
                    for pi in range(2):
                        tr = ps_attn.tile([P, P], BF16, tag="av", bufs=2,
                                          name="tr")
                        for j2 in range(2):
                            nc.tensor.transpose(
                                tr[j2 * HD:(j2 + 1) * HD, :],
                                yt_all[:, 2 * pi + j2, :], ident[:])
                        hp = 2 * g + pi   # feature chunk index 0..5
                        nc.vector.tensor_copy(
                            yT[hp // 2][:, hp % 2, qc * P:(qc + 1) * P], tr[:])

            # one-group lookahead, v-gemm chunks interleaved to fill the PE
            # gaps while ACT computes the first exps
            scores(0)
            v_chunk(0)
            v_chunk(1)
            scores(1)
            v_chunk(2)
            v_chunk(3)
            av_norm(0)
            scores(2)
            av_norm(1)
            av_norm(2)
            if "v66" in taps:
                for j, t in enumerate(v66):
                    nc.sync.dma_start(
                        taps["v66"][j * P:(j + 1) * P, :],
                        t[:].rearrange("p j h d -> p (j h d)"))
            dump_pairs("yT", yT)

            # ---- proj gemm + residual ----
            def resid_evac(scale):
                def ev(m, ps, r):
                    nc.vector.scalar_tensor_tensor(
                        out=xT[m][:], in0=ps[:], scalar=scale, in1=xT[m][:],
                        op0=OP.mult, op1=OP.add)
                return ev
            gemm_fm(wpr, l, D, yT, "wpr", D, resid_evac(PR_EVAC),
                    dma_q=lambda kp: nc.sync)
            if "xattn" in taps:
                for j, t in enumerate(xT):
                    nc.sync.dma_start(taps["xattn"][j * P:(j + 1) * P, :], t[:])

            h2 = layer_norm("ln2")
            dump_pairs("h2", h2)

            # ---- fc1 gemm + gelu ----
            gT = [None] * FKP

            def gelu_evac(m, ps, r):
                gp = gT[m // 2]
                if gp is None:
                    gp = acts.tile([P, 2, T], FP8, tag="gT", bufs=12)
                    gT[m // 2] = gp
                nc.scalar.activation(gp[:, m % 2, :], ps[:], AF.Gelu_apprx_tanh,
                                     scale=FC1_EVAC)
            gemm_fm(wfc, l, DF, h2, "wfc", 2 * 768, gelu_evac,
                    dma_q=lambda kp: nc.sync)
            dump_pairs("gT", gT)

            # ---- fc2 gemm + residual ----
            gemm_fm(wf2, l, D, gT, "wf2", D, resid_evac(FC2_EVAC), nkp=FKP,
                    dma_q=lambda kp: nc.sync)

        for l in range(n_layers):
            layer(l)
        if "xfinal" in taps:
            for j, t in enumerate(xT):
                nc.sync.dma_start(taps["xfinal"][j * P:(j + 1) * P, :], t[:])

        if not with_head:
            for j in range(DK):
                o = acts.tile([P, T], FP32, tag="o_nh", bufs=2)
                nc.vector.tensor_copy(o[:], xT[j][:])
                nc.sync.dma_start(out[j * P:(j + 1) * P, :], o[:])
        else:
            # ---- LM head: logits[t, v] = x @ wembT, bf16 out ----
            NV = 512
            for vs in range(0, V, NV):
                nn = min(NV, V - vs)
                w_sb = wpool.tile([P, DK, NV], BF16, tag="whead", bufs=2)
                nc.sync.dma_start(
                    w_sb[:].rearrange("p k v -> p (k v)"),
                    wembP[vs // NV, :, :])
                for tch in range(TCH):
                    ps = ps_gemm.tile([P, NV], FP32, tag="g", bufs=2)
                    for k in range(DK):
                        nc.tensor.matmul(
                            ps[:, :nn], xT[k][:, tch * P:(tch + 1) * P],
                            w_sb[:, k, :nn], start=(k == 0), stop=(k == DK - 1))
                    o = acts.tile([P, NV], BF16, tag="o_head", bufs=3)
                    if tch % 2 == 0:
                        nc.vector.tensor_copy(o[:, :nn], ps[:, :nn])
                    else:
                        nc.scalar.copy(o[:, :nn], ps[:, :nn])
                    nc.scalar.dma_start(out[tch * P:(tch + 1) * P, vs:vs + nn],
                                      o[:, :nn])

    nc.compile()
    return nc


# ---------------------------------------------------------------------------
# host side
# ---------------------------------------------------------------------------

B = 8
NCORES = 8


def _np_layer_norm(x, g, b, eps=1e-5):
    m = x.mean(-1, keepdims=True)
    v = x.var(-1, keepdims=True)
    return (x - m) / np.sqrt(v + eps) * g + b


def _pack_head(word_emb):
    # [V, D] -> per 512-vocab chunk: [P=ki, DK=ko, 512] with wT[d, v],
    # d = ko*128 + ki; flatten to [NVCH, P, DK*512] bf16 (zero-pad tail)
    wT = np.ascontiguousarray(word_emb.T)  # [D, V]
    nv = (V + 511) // 512
    pad = nv * 512 - V
    if pad:
        wT = np.concatenate([wT, np.zeros((D, pad), np.float32)], axis=1)
    w4 = wT.reshape(DK, P, nv, 512).transpose(2, 1, 0, 3)  # [nv, ki, ko, 512]
    return np.ascontiguousarray(w4.reshape(nv, P, DK * 512)).astype(NP_BF16)


def _q8(w, s):
    return np.clip(np.asarray(w, np.float32) * s, -224.0, 224.0).astype(NP_FP8)


def _prep_in_maps(inputs):
    ids = np.asarray(inputs["input_ids"]).astype(np.int64)
    tt = np.asarray(inputs["token_type_ids"]).astype(np.int64)
    x0 = (np.asarray(inputs["word_emb"], np.float32)[ids]
          + np.asarray(inputs["pos_emb"], np.float32)[None, :ids.shape[1], :]
          + np.asarray(inputs["type_emb"], np.float32)[tt])
    x0 = _np_layer_norm(x0, np.asarray(inputs["emb_ln_g"], np.float32),
                        np.asarray(inputs["emb_ln_b"], np.float32))
    mask = np.asarray(inputs["attention_mask"], np.float32)

    wqkv = np.asarray(inputs["wqkv"], np.float32)
    wfc_in = np.asarray(inputs["wfc"], np.float32)
    ln1_g = np.asarray(inputs["ln1_g"], np.float32)
    ln2_g = np.asarray(inputs["ln2_g"], np.float32)
    for name in ("bqkv", "bproj", "bfc", "bfc2", "ln1_b", "ln2_b"):
        assert np.abs(np.asarray(inputs[name])).max() == 0.0, (
            f"{name} is nonzero; this kernel folds only zero biases")
    wq_eff = wqkv * ln1_g[:, :, None]
    wf_eff = wfc_in * ln2_g[:, :, None]
    packed = dict(
        wqk=_q8(np.ascontiguousarray(wq_eff[:, :, :2 * D]), SW_QKV),
        wv=_q8(np.ascontiguousarray(wq_eff[:, :, 2 * D:]), SW_QKV),
        wpr=_q8(inputs["wproj"], SW_PR),
        wfc=_q8(wf_eff, SW_QKV),
        wf2=_q8(inputs["wfc2"], SW_PR),
        wembP=_pack_head(np.asarray(inputs["word_emb"], np.float32)),
        ident=np.eye(P, dtype=NP_BF16),
    )
    in_maps = []
    for b in range(B):
        bias = -10000.0 * (1.0 - mask[b]) + np.log(SP)
        m = dict(packed)
        m["x0T"] = np.ascontiguousarray(x0[b].T).astype(NP_BF16)
        m["mb"] = np.ascontiguousarray(
            bias.reshape(TCH, P).T).astype(np.float32)
        in_maps.append(m)
    return in_maps


_NC_CACHE = {}


def get_nc():
    if "nc" not in _NC_CACHE:
        _NC_CACHE["nc"] = build(n_layers=12, with_head=True)
    return _NC_CACHE["nc"]


def kernel(**inputs) -> np.ndarray:
    nc = get_nc()
    in_maps = _prep_in_maps(inputs)
    res = bass_utils.run_bass_kernel_spmd(nc, in_maps, core_ids=list(range(NCORES)))
    return np.stack([res.results[b]["out"] for b in range(B)]).astype(np.float32)


# revision 32
# speedup vs baseline: 12.9375x; 12.9375x over previous
"""BERT-base (12-layer, B=8, T=512, D=768) forward + tied-embedding LM head
on 8 Trainium2 NeuronCores.

Sharding: data-parallel over the batch dimension -- core b computes batch
element b end-to-end (no collectives).

v2: fp8e4 DoubleRow matmuls for the five big GEMMs (qk, v, proj, fc1, fc2)
with power-of-two scales folded into evacuation ops; attention uses a
transposed AV formulation (out = p.T @ [ones|v], full-128 contraction) so
softmax normalization is a per-partition DVE reciprocal + scalar multiply --
no Ln/Exp act-table thrash, no DRAM broadcast round-trips. Residual stream
kept in bf16 so LN stat matmuls run at 1 cycle/row. Logits emitted bf16.
"""

from contextlib import ExitStack

import numpy as np
import ml_dtypes

import concourse.bass as bass
import concourse.bacc as bacc
import concourse.mybir as mybir
import concourse.tile as tile
from concourse import bass_utils
from concourse._compat import get_trn_type

NP_BF16 = ml_dtypes.bfloat16
NP_FP8 = ml_dtypes.float8_e4m3

FP32 = mybir.dt.float32
BF16 = mybir.dt.bfloat16
FP8 = mybir.dt.float8e4
AF = mybir.ActivationFunctionType
OP = mybir.AluOpType
DR = mybir.MatmulPerfMode.DoubleRow

P = 128
T = 512
D = 768
H = 12
HD = 64
DF = 3072
V = 30522
DK = D // P       # 6 feature chunks
KP = DK // 2      # 3 DoubleRow k-pairs over D
FKP = DF // 256   # 12 DoubleRow k-pairs over DF
TCH = T // P      # 4 token chunks
EPS = 1e-5

# power-of-two quantization scales
SA = 16.0         # LN output (h) fp8 scale
SW_QKV = 64.0     # wqk / wv / wfc weight scale
SW_PR = 256.0     # wproj / wfc2 weight scale
SV = 32.0         # v fp8 scale (folded: v8 = psum * SV/(SA*SW_QKV))
SP = 16.0         # softmax exp fp8 scale (folded into exp bias as ln SP)
EXP_SCALE = 0.125 / (SA * SW_QKV) ** 2       # 2^-23
V_EVAC = SV / (SA * SW_QKV)                  # 2^-5
PR_EVAC = 1.0 / (SV * SW_PR)                 # 2^-13
FC1_EVAC = 1.0 / (SA * SW_QKV)               # 2^-10
FC2_EVAC = 1.0 / SW_PR                       # 2^-8
LN_SA_BIAS = float(np.log(SA))


def build(n_layers=12, with_head=True, debug_taps=()):
    nc = bacc.Bacc(get_trn_type() or "TRN2", target_bir_lowering=False, debug=False)

    x0T = nc.dram_tensor("x0T", [D, T], BF16, kind="ExternalInput")
    mb = nc.dram_tensor("mb", [P, TCH], FP32, kind="ExternalInput")
    ident_d = nc.dram_tensor("ident", [P, P], BF16, kind="ExternalInput")
    L = max(n_layers, 1)
    wqk = nc.dram_tensor("wqk", [L, D, 2 * D], FP8, kind="ExternalInput")
    wv = nc.dram_tensor("wv", [L, D, D], FP8, kind="ExternalInput")
    wpr = nc.dram_tensor("wpr", [L, D, D], FP8, kind="ExternalInput")
    wfc = nc.dram_tensor("wfc", [L, D, DF], FP8, kind="ExternalInput")
    wf2 = nc.dram_tensor("wf2", [L, DF, D], FP8, kind="ExternalInput")
    if with_head:
        NVCH = (V + 511) // 512
        wembP = nc.dram_tensor("wembP", [NVCH, P, DK * 512], BF16,
                               kind="ExternalInput")
        out = nc.dram_tensor("out", [T, V], BF16, kind="ExternalOutput")
    else:
        out = nc.dram_tensor("out", [D, T], FP32, kind="ExternalOutput")

    tap_specs = {
        "h1": ([KP * P, 2 * T], FP8), "qkT": ([2 * D, T], BF16),
        "v66": ([2 * P, 2 * H * 66], FP8), "p0": ([P, 4 * T], FP8),
        "yT": ([KP * P, 2 * T], FP8), "xattn": ([D, T], BF16),
        "h2": ([KP * P, 2 * T], FP8), "gT": ([FKP * P, 2 * T], FP8),
        "xfinal": ([D, T], BF16),
    }
    taps = {}
    for name in debug_taps:
        shape, dt = tap_specs[name]
        taps[name] = nc.dram_tensor(f"tap_{name}", shape, dt, kind="ExternalOutput")

    with tile.TileContext(nc) as tc, ExitStack() as ctx:
        consts = ctx.enter_context(tc.tile_pool(name="consts", bufs=1))
        resid = ctx.enter_context(tc.tile_pool(name="resid", bufs=1))
        acts = ctx.enter_context(tc.tile_pool(name="acts", bufs=2))
        wpool = ctx.enter_context(tc.tile_pool(name="wpool", bufs=1))
        small = ctx.enter_context(tc.tile_pool(name="small", bufs=2))
        ps_ln = ctx.enter_context(tc.tile_pool(name="ps_ln", bufs=1, space="PSUM"))
        ps_gemm = ctx.enter_context(tc.tile_pool(name="ps_gemm", bufs=1, space="PSUM"))
        ps_attn = ctx.enter_context(tc.tile_pool(name="ps_attn", bufs=1, space="PSUM"))

        czero = consts.tile([P, 1], FP32, tag="czero")
        nc.vector.memset(czero[:], 0.0)
        ceps = consts.tile([P, 1], FP32, tag="ceps")
        nc.vector.memset(ceps[:], EPS)
        nc.const_aps.aps[(FP32, 0.0)] = czero[:]
        nc.const_aps.aps[(FP32, EPS)] = ceps[:]
        csa = consts.tile([P, 1], FP32, tag="csa")
        nc.vector.memset(csa[:], LN_SA_BIAS)
        nc.const_aps.aps[(FP32, LN_SA_BIAS)] = csa[:]

        ones_bf = consts.tile([P, 1], BF16, tag="ones_bf")
        nc.vector.memset(ones_bf[:], 1.0)
        ones_row = consts.tile([1, P], BF16, tag="ones_row")
        nc.vector.memset(ones_row[:], 1.0)
        mb_sb = consts.tile([P, TCH], FP32, tag="mb_sb")
        nc.sync.dma_start(mb_sb[:], mb[:])
        ident = consts.tile([P, P], BF16, tag="ident")
        nc.sync.dma_start(ident[:], ident_d[:])

        ones_f32 = consts.tile([P, 1], FP32, tag="ones_f32")
        nc.vector.memset(ones_f32[:], 1.0)
        dummy_in = consts.tile([1, 1], FP32, tag="dummy_in")
        nc.vector.memset(dummy_in[:], 1.0)

        def preload_ln_table():
            # throwaway Ln: pulls the natural_log table load off the next
            # LayerNorm's critical chain (runs during surrounding compute)
            dsink = small.tile([1, 1], FP32, tag="dsink", bufs=2)
            nc.scalar.activation(dsink[:], dummy_in[:], AF.Ln)
        F32R = mybir.dt.float32r

        # residual stream, bf16 feature-major [D, T] in 6 chunks
        xT = []
        for j in range(DK):
            t = resid.tile([P, T], BF16, tag=f"x{j}")
            nc.sync.dma_start(t[:], x0T[j * P:(j + 1) * P, :])
            xT.append(t)

        def layer_norm(tag):
            """LN over xT -> 3 fp8 pair-tiles [P, 2, T], values scaled by SA"""
            sum_ps = ps_ln.tile([1, T], FP32, tag="stat", bufs=2, name="sum_ps")
            ssq_ps = ps_ln.tile([1, T], FP32, tag="stat", bufs=2, name="ssq_ps")
            sq_tiles = []
            for j in range(DK):
                sq = acts.tile([P, T], BF16, tag="sq", bufs=6)
                if j % 2 == 0:
                    nc.scalar.activation(sq[:], xT[j][:], AF.Square)
                else:
                    nc.gpsimd.tensor_mul(sq[:], xT[j][:], xT[j][:])
                sq_tiles.append(sq)
                nc.tensor.matmul(sum_ps[:], ones_bf[:], xT[j][:],
                                 start=(j == 0), stop=(j == DK - 1))
            for j in range(DK):
                nc.tensor.matmul(ssq_ps[:], ones_bf[:], sq_tiles[j][:],
                                 start=(j == 0), stop=(j == DK - 1))
            nm = small.tile([1, T], FP32, tag="srow", bufs=4, name="nm")
            nc.vector.tensor_scalar_mul(nm[:], sum_ps[:], -1.0 / D)
            msq = small.tile([1, T], FP32, tag="srow", bufs=4, name="msq")
            nc.vector.tensor_mul(msq[:], nm[:], nm[:])
            var = small.tile([1, T], FP32, tag="srow", bufs=4, name="var")
            nc.vector.scalar_tensor_tensor(
                out=var[:], in0=ssq_ps[:], scalar=1.0 / D, in1=msq[:],
                op0=OP.mult, op1=OP.subtract)
            lnv = small.tile([1, T], FP32, tag="srow", bufs=4, name="lnv")
            nc.scalar.activation(lnv[:], var[:], AF.Ln, bias=EPS)
            # SA * rsqrt(var): exp(-0.5*ln(v) + ln SA), bf16 out
            rstd = small.tile([1, T], BF16, tag="brow", bufs=4, name="rstd")
            nc.scalar.activation(rstd[:], lnv[:], AF.Exp, scale=-0.5,
                                 bias=LN_SA_BIAS)
            nmrs = small.tile([1, T], BF16, tag="brow", bufs=4, name="nmrs")
            nc.vector.tensor_mul(nmrs[:], nm[:], rstd[:])
            rb = ps_ln.tile([P, T], FP32, tag="stat", bufs=2, name="rb")
            nc.tensor.matmul(rb[:], ones_row[:], rstd[:], start=True, stop=True)
            nb = ps_ln.tile([P, T], FP32, tag="stat", bufs=2, name="nb")
            nc.tensor.matmul(nb[:], ones_row[:], nmrs[:], start=True, stop=True)
            h_pairs = []
            for jp in range(KP):
                hp = acts.tile([P, 2, T], FP8, tag=f"h_{tag}", bufs=4)
                for j2 in range(2):
                    j = 2 * jp + j2
                    t1 = acts.tile([P, T], BF16, tag="lnt", bufs=3)
                    nc.vector.tensor_mul(t1[:], xT[j][:], rb[:])
                    nc.vector.tensor_add(hp[:, j2, :], t1[:], nb[:])
                h_pairs.append(hp)
            return h_pairs

        def gemm_fm(w3, l, M, rhs_pairs, tag, CG, evac, nkp=KP, evac_rot=1,
                    dma_q=None):
            """feature-major DoubleRow GEMM: psum[128,T] = sum_kp
            w3[l, kp-pair, m-chunk].T @ rhs_pairs[kp]"""
            ei = 0
            for cg0 in range(0, M, CG):
                cgn = min(CG, M - cg0)
                slabs = []
                for kp in range(nkp):
                    s = wpool.tile([P, 2, CG], FP8, tag=f"{tag}_{kp}", bufs=2)
                    eng = dma_q(kp) if dma_q else nc.sync
                    eng.dma_start(
                        s[:, :, :cgn],
                        w3[l, kp * 256:(kp + 1) * 256, cg0:cg0 + cgn]
                        .rearrange("(j p) c -> p j c", p=P))
                    slabs.append(s)
                for mi in range(cgn // P):
                    m = (cg0 // P) + mi
                    ps = ps_gemm.tile([P, T], FP32, tag="g", bufs=2)
                    for kp in range(nkp):
                        nc.tensor.matmul(ps[:], slabs[kp][:, :, mi * P:(mi + 1) * P],
                                         rhs_pairs[kp][:],
                                         start=(kp == 0), stop=(kp == nkp - 1),
                                         perf_mode=DR)
                    evac(m, ps, ei % evac_rot)
                    ei += 1

        def dump_pairs(name, tiles):
            if name in taps:
                for j, t in enumerate(tiles):
                    nc.sync.dma_start(
                        taps[name][j * P:(j + 1) * P, :],
                        t[:].rearrange("p j t -> p (j t)"))

        def layer(l):
            h1 = layer_norm("ln1")
            dump_pairs("h1", h1)

            # ---- QK gemm: qkT[c, t] raw (scaled by SA*SW_QKV), bf16 ----
            qkT = [None] * (2 * D // P)

            def qk_evac(m, ps, r):
                qt = acts.tile([P, T], BF16, tag="qkT", bufs=12)
                if r == 0:
                    nc.scalar.copy(qt[:], ps[:])
                else:
                    nc.vector.tensor_copy(qt[:], ps[:])
                qkT[m] = qt
            gemm_fm(wqk, l, 2 * D, h1, "wqk", 2 * D, qk_evac, evac_rot=2,
                    dma_q=lambda kp: nc.sync)
            if "qkT" in taps:
                for j, t in enumerate(qkT):
                    nc.sync.dma_start(taps["qkT"][j * P:(j + 1) * P, :], t[:])

            # ---- V gemm (token-major): v66[kcp][p, j, h, 0]=1, [.,1:65]=v*SV
            v_slabs = []
            for kp in range(KP):
                s = wpool.tile([P, 2, D], FP8, tag=f"wv_{kp}", bufs=2)
                nc.sync.dma_start(
                    s[:], wv[l, kp * 256:(kp + 1) * 256, :]
                    .rearrange("(j p) c -> p j c", p=P))
                v_slabs.append(s)
            v66 = []
            for kcp in range(2):
                vt = acts.tile([P, 2, H, 66], FP8, tag="v66", bufs=4)
                nc.vector.memset(vt[:, :, :, 0:1], 1.0)
                v66.append(vt)

            def v_chunk(tch):
                for n0 in range(0, D, T):  # {0:512, 512:256}
                    nn = min(T, D - n0)
                    ps = ps_gemm.tile([P, T], FP32, tag="g", bufs=2)
                    for kp in range(KP):
                        nc.tensor.matmul(
                            ps[:, :nn],
                            h1[kp][:, :, tch * P:(tch + 1) * P],
                            v_slabs[kp][:, :, n0:n0 + nn],
                            start=(kp == 0), stop=(kp == KP - 1), perf_mode=DR)
                    dst = v66[tch // 2][:, tch % 2, n0 // HD:(n0 + nn) // HD, 1:65]
                    src = ps[:, :nn].rearrange("p (h d) -> p h d", d=HD)
                    if n0 == 0:
                        nc.scalar.activation(dst, src, AF.Copy, scale=V_EVAC)
                    else:
                        nc.vector.tensor_scalar_mul(dst, src, V_EVAC)

            # ---- attention: p = exp(k.T q * s + b) in fp8; y via p.T@[1|v]
            yT = [acts.tile([P, 2, T], FP8, tag="yT", bufs=4, name=f"yT{i}")
                  for i in range(KP)]
            p_tiles = {}

            def scores(g):
                for hh in range(4):
                    h = 4 * g + hh
                    ht, r = h // 2, h % 2
                    rows = slice(r * HD, (r + 1) * HD)
                    qt = qkT[ht]
                    kt = qkT[DK + ht]
                    for kcp in range(2):
                        pt = acts.tile([P, 2, T], FP8, tag="p", bufs=12)
                        for j2 in range(2):
                            kc = 2 * kcp + j2
                            s_ps = ps_attn.tile([P, T], FP32, tag="sc", bufs=2)
                            nc.tensor.matmul(s_ps[:], kt[rows, kc * P:(kc + 1) * P],
                                             qt[rows, :], start=True, stop=True)
                            nc.scalar.activation(pt[:, j2, :], s_ps[:], AF.Exp,
                                                 bias=mb_sb[:, kc:kc + 1],
                                                 scale=EXP_SCALE)
                        p_tiles[(h, kcp)] = pt
                if g == 0 and "p0" in taps:
                    for kcp in range(2):
                        nc.sync.dma_start(
                            taps["p0"][:, kcp * 2 * T:(kcp + 1) * 2 * T],
                            p_tiles[(0, kcp)][:].rearrange("p j t -> p (j t)"))

            def av_norm(g):
                for qc in range(TCH):
                    av = ps_attn.tile([P, 4, 66], FP32, tag="av", bufs=2,
                                      name="av")
                    for hh in range(4):
                        h = 4 * g + hh
                        for kcp in range(2):
                            nc.tensor.matmul(
                                av[:, hh, :],
                                p_tiles[(h, kcp)][:, :, qc * P:(qc + 1) * P],
                                v66[kcp][:, :, h, :],
                                start=(kcp == 0), stop=(kcp == 1), perf_mode=DR)
                    rec = small.tile([P, 4, 1], FP32, tag="rec", bufs=4)
                    nc.vector.reciprocal_approx_fast(rec[:], av[:, :, 0:1])
                    # one batched multiply: yt_all[p,hh,d] = av[p,hh,1+d]*rec[p,hh]
                    yt_all = acts.tile([P, 4, HD], BF16, tag="yt", bufs=4)
                    rec_b = bass.AP(tensor=rec.tensor, offset=rec.offset,
                                    ap=[rec[:].ap[0], [1, 4], [0, HD]])
                    nc.vector.tensor_mul(yt_all[:], av[:, :, 1:65], rec_b)
                    for pi in range(2):
                        tr = ps_attn.tile([P, P], BF16, tag="av", bufs=2,
                                          name="tr")
                        for j2 in range(2):
                            nc.tensor.transpose(
                                tr[j2 * HD:(j2 + 1) * HD, :],
                                yt_all[:, 2 * pi + j2, :], ident[:])
                        hp = 2 * g + pi   # feature chunk index 0..5
                        nc.vector.tensor_copy(
                            yT[hp // 2][:, hp % 2, qc * P:(qc + 1) * P], tr[:])

            # one-group lookahead, v-gemm chunks interleaved to fill the PE
            # gaps while ACT computes the first exps
            scores(0)
            v_chunk(0)
            v_chunk(1)
            scores(1)
            v_chunk(2)
            v_chunk(3)
            av_norm(0)
            scores(2)
            av_norm(1)
            av_norm(2)
            preload_ln_table()
            if "v66" in taps:
                for j, t in enumerate(v66):
                    nc.sync.dma_start(
                        taps["v66"][j * P:(j + 1) * P, :],
                        t[:].rearrange("p j h d -> p (j h d)"))
            dump_pairs("yT", yT)

            # ---- proj gemm + residual ----
            def resid_evac(scale):
                def ev(m, ps, r):
                    nc.vector.scalar_tensor_tensor(
                        out=xT[m][:], in0=ps[:], scalar=scale, in1=xT[m][:],
                        op0=OP.mult, op1=OP.add)
                return ev
            gemm_fm(wpr, l, D, yT, "wpr", D, resid_evac(PR_EVAC),
                    dma_q=lambda kp: nc.sync)
            if "xattn" in taps:
                for j, t in enumerate(xT):
                    nc.sync.dma_start(taps["xattn"][j * P:(j + 1) * P, :], t[:])

            h2 = layer_norm("ln2")
            dump_pairs("h2", h2)

            # ---- fc1 gemm + gelu ----
            gT = [None] * FKP

            def gelu_evac(m, ps, r):
                gp = gT[m // 2]
                if gp is None:
                    gp = acts.tile([P, 2, T], FP8, tag="gT", bufs=12)
                    gT[m // 2] = gp
                nc.scalar.activation(gp[:, m % 2, :], ps[:], AF.Gelu_apprx_tanh,
                                     scale=FC1_EVAC)
            gemm_fm(wfc, l, DF, h2, "wfc", 2 * 768, gelu_evac,
                    dma_q=lambda kp: nc.sync)
            dump_pairs("gT", gT)

            # ---- fc2 gemm + residual ----
            preload_ln_table()
            gemm_fm(wf2, l, D, gT, "wf2", D, resid_evac(FC2_EVAC), nkp=FKP,
                    dma_q=lambda kp: nc.sync)

        for l in range(n_layers):
            layer(l)
        if "xfinal" in taps:
            for j, t in enumerate(xT):
                nc.sync.dma_start(taps["xfinal"][j * P:(j + 1) * P, :], t[:])

        if not with_head:
            for j in range(DK):
                o = acts.tile([P, T], FP32, tag="o_nh", bufs=2)
                nc.vector.tensor_copy(o[:], xT[j][:])
                nc.sync.dma_start(out[j * P:(j + 1) * P, :], o[:])
        else:
            # ---- LM head: logits[t, v] = x @ wembT, bf16 out ----
            NV = 512
            for vs in range(0, V, NV):
                nn = min(NV, V - vs)
                w_sb = wpool.tile([P, DK, NV], BF16, tag="whead", bufs=2)
                nc.sync.dma_start(
                    w_sb[:].rearrange("p k v -> p (k v)"),
                    wembP[vs // NV, :, :])
                for tch in range(TCH):
                    ps = ps_gemm.tile([P, NV], FP32, tag="g", bufs=2)
                    for k in range(DK):
                        nc.tensor.matmul(
                            ps[:, :nn], xT[k][:, tch * P:(tch + 1) * P],
                            w_sb[:, k, :nn], start=(k == 0), stop=(k == DK - 1))
                    o = acts.tile([P, NV], BF16, tag="o_head", bufs=3)
                    if tch % 2 == 0:
                        nc.vector.tensor_copy(o[:, :nn], ps[:, :nn])
                    else:
                        nc.scalar.copy(o[:, :nn], ps[:, :nn])
                    nc.scalar.dma_start(out[tch * P:(tch + 1) * P, vs:vs + nn],
                                      o[:, :nn])

    nc.compile()
    return nc


# ---------------------------------------------------------------------------
# host side
# ---------------------------------------------------------------------------

B = 8
NCORES = 8


def _np_layer_norm(x, g, b, eps=1e-5):
    m = x.mean(-1, keepdims=True)
    v = x.var(-1, keepdims=True)
    return (x - m) / np.sqrt(v + eps) * g + b


def _pack_head(word_emb):
    # [V, D] -> per 512-vocab chunk: [P=ki, DK=ko, 512] with wT[d, v],
    # d = ko*128 + ki; flatten to [NVCH, P, DK*512] bf16 (zero-pad tail)
    wT = np.ascontiguousarray(word_emb.T)  # [D, V]
    nv = (V + 511) // 512
    pad = nv * 512 - V
    if pad:
        wT = np.concatenate([wT, np.zeros((D, pad), np.float32)], axis=1)
    w4 = wT.reshape(DK, P, nv, 512).transpose(2, 1, 0, 3)  # [nv, ki, ko, 512]
    return np.ascontiguousarray(w4.reshape(nv, P, DK * 512)).astype(NP_BF16)


def _q8(w, s):
    return np.clip(np.asarray(w, np.float32) * s, -224.0, 224.0).astype(NP_FP8)


def _prep_in_maps(inputs):
    ids = np.asarray(inputs["input_ids"]).astype(np.int64)
    tt = np.asarray(inputs["token_type_ids"]).astype(np.int64)
    x0 = (np.asarray(inputs["word_emb"], np.float32)[ids]
          + np.asarray(inputs["pos_emb"], np.float32)[None, :ids.shape[1], :]
          + np.asarray(inputs["type_emb"], np.float32)[tt])
    x0 = _np_layer_norm(x0, np.asarray(inputs["emb_ln_g"], np.float32),
                        np.asarray(inputs["emb_ln_b"], np.float32))
    mask = np.asarray(inputs["attention_mask"], np.float32)

    wqkv = np.asarray(inputs["wqkv"], np.float32)
    wfc_in = np.asarray(inputs["wfc"], np.float32)
    ln1_g = np.asarray(inputs["ln1_g"], np.float32)
    ln2_g = np.asarray(inputs["ln2_g"], np.float32)
    for name in ("bqkv", "bproj", "bfc", "bfc2", "ln1_b", "ln2_b"):
        assert np.abs(np.asarray(inputs[name])).max() == 0.0, (
            f"{name} is nonzero; this kernel folds only zero biases")
    wq_eff = wqkv * ln1_g[:, :, None]
    wf_eff = wfc_in * ln2_g[:, :, None]
    packed = dict(
        wqk=_q8(np.ascontiguousarray(wq_eff[:, :, :2 * D]), SW_QKV),
        wv=_q8(np.ascontiguousarray(wq_eff[:, :, 2 * D:]), SW_QKV),
        wpr=_q8(inputs["wproj"], SW_PR),
        wfc=_q8(wf_eff, SW_QKV),
        wf2=_q8(inputs["wfc2"], SW_PR),
        wembP=_pack_head(np.asarray(inputs["word_emb"], np.float32)),
        ident=np.eye(P, dtype=NP_BF16),
    )
    in_maps = []
    for b in range(B):
        bias = -10000.0 * (1.0 - mask[b]) + np.log(SP)
        m = dict(packed)
        m["x0T"] = np.ascontiguousarray(x0[b].T).astype(NP_BF16)
        m["mb"] = np.ascontiguousarray(
            bias.reshape(TCH, P).T).astype(np.float32)
        in_maps.append(m)
    return in_maps


_NC_CACHE = {}


def get_nc():
    if "nc" not in _NC_CACHE:
        _NC_CACHE["nc"] = build(n_layers=12, with_head=True)
    return _NC_CACHE["nc"]


def kernel(**inputs) -> np.ndarray:
    nc = get_nc()
    in_maps = _prep_in_maps(inputs)
    res = bass_utils.run_bass_kernel_spmd(nc, in_maps, core_ids=list(range(NCORES)))
    return np.stack([res.results[b]["out"] for b in range(B)]).astype(np.float32)
